# revision 1
# baseline (speedup 1.0000x reference)
"""DCL loss on Trainium2, 8 cores — v2: triangle symmetry for sim00/sim11.

sim00 and sim11 are symmetric, so each unordered block pair (b,b') needs
exp() only once: its exp'd block contributes a row-sum to block b and a
column-sum (via PE ones-matmul) to block b'.  Block rows (512 rows each,
nb = N/512 of them) are paired {r, nb-1-r} per core for load balance.
Block-row b computes column blocks {b..b+nb/2-1} mod nb, plus b+nb/2 iff
b < nb/2 — every pair covered exactly once, and per-core work is equal.
The per-core mod-window inputs are materialized by the host (pure data
movement), so the SPMD program stays fully static.  Cuts exp() work (the
scalar-engine bottleneck) from 3*N^2 to 2*N^2 elements.

Everything else as v1.6: l2-normalize on device (rsqrt via exp(-ln/2) to
stay on one ACT table set), bf16 PE grams, exp from PSUM with fused
row-sum accumulation, col-group-packed ones-matmul column sums, O(N)
host combine.
"""

import numpy as np

import concourse.bass as bass
import concourse.tile as tile
from concourse import bacc, mybir
from concourse.bass_utils import run_bass_kernel_spmd
from concourse.masks import make_identity

F32 = mybir.dt.float32
BF16 = mybir.dt.bfloat16
AF = mybir.ActivationFunctionType

N_TOTAL = 8192
C = 128
N_CORES = 8
INV_T = 10.0
CHUNK = 1536
BLK = 512


def _chunks(width, chunk=CHUNK):
    out = []
    s = 0
    while s < width:
        w = min(chunk, width - s)
        out.append((s, w))
        s += w
    return out


def _layout(n_total):
    nb = n_total // BLK
    wA = (nb // 2 + 1) * BLK
    wB = (nb // 2) * BLK
    cA, cB, c01 = _chunks(wA), _chunks(wB), _chunks(n_total)
    segs = {}
    base = 0
    for seg, ncs, mc in (("xA", len(cA), 4), ("xB", len(cB), 4),
                         ("yA", len(cA), 4), ("yB", len(cB), 4),
                         ("01", len(c01), 8)):
        segs[seg] = (base, ncs, mc)
        base += ncs * mc
    return nb, wA, wB, cA, cB, c01, segs, base


def build(n_total=N_TOTAL, n_cores=N_CORES):
    P = 128
    nb, wA, wB, cA, cB, c01, segs, rcols = _layout(n_total)
    assert nb == 2 * n_cores

    nc = bacc.Bacc("TRN2", target_bir_lowering=False, debug=False,
                   num_devices=n_cores)

    din = {}
    for k, w in (("xwA", wA), ("xwB", wB), ("ywA", wA), ("ywB", wB),
                 ("yf", n_total)):
        din[k] = nc.dram_tensor(k, [w, C], F32, kind="ExternalInput").ap()

    d_rowsums = nc.dram_tensor("rowsums", [P, rcols], F32,
                               kind="ExternalOutput").ap()
    d_colsums01 = nc.dram_tensor("colsums01", [1, n_total], F32,
                                 kind="ExternalOutput").ap()
    d_colsyms = nc.dram_tensor("colsyms", [4, wA - BLK], F32,
                               kind="ExternalOutput").ap()
    d_diags = nc.dram_tensor("diags", [3, 2 * BLK], F32,
                             kind="ExternalOutput").ap()

    widths = {"xwA": wA, "xwB": wB, "ywA": wA, "ywB": wB, "yf": n_total}

    with tile.TileContext(nc) as tc:
        with (
            tc.tile_pool(name="big", bufs=1) as big,
            tc.tile_pool(name="work", bufs=4) as work,
            tc.tile_pool(name="expb", bufs=6) as expb,
            tc.tile_pool(name="sim", bufs=2, space="PSUM") as simp,
            tc.tile_pool(name="misc", bufs=2, space="PSUM") as miscp,
        ):
            ident = big.tile([P, P], BF16, tag="ident")
            make_identity(nc, ident)
            ones_b = big.tile([P, 1], BF16, tag="ones")
            nc.vector.memset(ones_b, 1.0)
            ones_f = big.tile([P, 1], F32, tag="ones_f")
            nc.vector.memset(ones_f, 1.0)

            T, rsq = {}, {}
            for k, w in widths.items():
                T[k] = big.tile([P, w], BF16, tag=f"T_{k}", name=f"T_{k}")

            rows_sb = big.tile([P, rcols], F32, tag="rows_sb")
            SLAB = 8

            def stats(key):
                """pass 1: stream tiles, accumulate sumsq, compute rsqrt."""
                w = widths[key]
                tiles = w // P
                src3 = din[key].rearrange("(t p) c -> p t c", p=P)
                ss = big.tile([P, tiles], F32, tag=f"ss_{key}", name=f"ss_{key}")
                rs = big.tile([P, tiles], F32, tag=f"rs_{key}", name=f"rs_{key}")
                rsq[key] = rs
                for s in range(0, tiles, SLAB):
                    se = min(s + SLAB, tiles)
                    ld = work.tile([P, SLAB, C], F32, tag="ld1")
                    nc.sync.dma_start(out=ld[:, : se - s, :],
                                      in_=src3[:, s:se, :])
                    sq = work.tile([P, SLAB, C], F32, tag="sq")
                    nc.vector.tensor_mul(sq[:, : se - s, :], ld[:, : se - s, :],
                                         ld[:, : se - s, :])
                    nc.vector.reduce_sum(out=ss[:, s:se], in_=sq[:, : se - s, :],
                                         axis=mybir.AxisListType.X)
                lg = work.tile([P, tiles], F32, tag="lg")
                nc.scalar.activation(out=lg, in_=ss, func=AF.Ln)
                nc.scalar.activation(out=rs, in_=lg, func=AF.Exp, scale=-0.5)

            def ntp(key, lo=0, hi=None):
                """pass 2: stream tiles again, normalize bf16, PE-transpose."""
                w = widths[key]
                tiles = w // P
                if hi is None:
                    hi = tiles
                src3 = din[key].rearrange("(t p) c -> p t c", p=P)
                rs = rsq[key]
                for s in range(lo, hi, SLAB):
                    se = min(s + SLAB, hi)
                    ld = work.tile([P, SLAB, C], F32, tag="ld2")
                    nc.sync.dma_start(out=ld[:, : se - s, :],
                                      in_=src3[:, s:se, :])
                    nrm = work.tile([P, SLAB, C], BF16, tag="nrm")
                    rs_sl = rs[:, s:se]
                    rs_b = bass.AP(tensor=rs_sl.tensor, offset=rs_sl.offset,
                                   ap=[rs_sl.ap[0], rs_sl.ap[1], [0, C]])
                    nc.vector.tensor_mul(nrm[:, : se - s, :],
                                         ld[:, : se - s, :], rs_b)
                    for t in range(s, se):
                        grp = t % 4
                        if grp == 0:
                            pt = miscp.tile([P, 4 * P], BF16, tag="misc",
                                            name=f"pt_{key}_{t}")
                        nc.tensor.transpose(pt[:, grp * P:(grp + 1) * P],
                                            nrm[:, t - s, :], ident)
                        if grp == 3 or t == tiles - 1:
                            ww = (grp + 1) * P
                            dst = T[key][:, (t - grp) * P:(t - grp) * P + ww]
                            nc.vector.tensor_copy(out=dst, in_=pt[:, :ww])

            def gram(seg, akey, bkey, chunks, mcount, colsum_dram, col_off):
                """rows = T[akey][:, 0:mcount*128] x cols T[bkey][:, :width].

                Row sums via ACT accum.  Column sums via col-group-packed
                ones-matmuls accumulated in PSUM over the row tiles, for
                512-slices at global position >= col_off (skips the diag
                block for symmetric grams).  colsum_dram row gets the
                partial sums at [global_pos - col_off].
                """
                base, ncs, mc = segs[seg]
                assert mc == mcount and ncs == len(chunks)
                for ci, (cs, cw) in enumerate(chunks):
                    slices = [s for s in range(cw // 512)
                              if cs + s * 512 >= col_off]
                    if slices:
                        cp = miscp.tile([P, 512], F32, tag="misc",
                                        name=f"cp_{seg}_{ci}")

                    def emit_colsums(m, eb):
                        for gi, s in enumerate(slices):
                            nc.tensor.matmul(
                                cp[32 * gi:32 * gi + 1, :], ones_b,
                                eb[:, s * 512:(s + 1) * 512],
                                start=(m == 0), stop=(m == mcount - 1),
                                tile_position=(0, 32 * gi),
                                skip_group_check=True)

                    pend = None  # (m, eb): colsums lag one row tile so the
                    # next tile's matmuls are queued before PE blocks on exp
                    for m in range(mcount):
                        if mcount == 8:  # sim01: rows from both window prefixes
                            kk = akey if m < 4 else akey.replace("A", "B")
                            lhsT = T[kk][:, (m % 4) * P:(m % 4) * P + P]
                        else:
                            lhsT = T[akey][:, m * P:(m + 1) * P]
                        ps = simp.tile([P, CHUNK], F32, tag="sim")
                        for s in range(0, cw, 512):
                            nc.tensor.matmul(ps[:, s:s + 512], lhsT,
                                             T[bkey][:, cs + s:cs + s + 512],
                                             start=True, stop=True)
                        if pend is not None:
                            emit_colsums(*pend)
                        eb = expb.tile([P, CHUNK], BF16, tag="eb",
                                       name=f"eb_{seg}_{ci}_{m}")
                        col = base + m * ncs + ci
                        nc.scalar.activation(out=eb[:, :cw], in_=ps[:, :cw],
                                             func=AF.Exp, scale=INV_T,
                                             accum_out=rows_sb[:, col:col + 1])
                        pend = (m, eb)
                    if pend is not None:
                        emit_colsums(*pend)
                    if slices:
                        csb = work.tile([1, CHUNK], F32, tag="csb")
                        for gi, s in enumerate(slices):
                            nc.vector.tensor_copy(
                                out=csb[0:1, gi * 512:(gi + 1) * 512],
                                in_=cp[32 * gi:32 * gi + 1, :])
                        w0 = cs + slices[0] * 512 - col_off
                        nc.sync.dma_start(
                            out=colsum_dram[0:1, w0:w0 + len(slices) * 512],
                            in_=csb[0:1, :len(slices) * 512])

            def diag_block():
                for row, (a, b) in enumerate((("xw", "xw"), ("xw", "yw"),
                                              ("yw", "yw"))):
                    for pi, part in enumerate(("A", "B")):
                        prod = work.tile([P, BLK], F32, tag="diagprod")
                        nc.vector.tensor_mul(prod, T[a + part][:, :BLK],
                                             T[b + part][:, :BLK])
                        dp = miscp.tile([1, 512], F32, tag="misc")
                        nc.tensor.matmul(dp, ones_f, prod, start=True,
                                         stop=True)
                        dsb = work.tile([1, 512], F32, tag="dsb")
                        nc.vector.tensor_copy(out=dsb, in_=dp)
                        nc.sync.dma_start(
                            out=d_diags[row:row + 1,
                                        pi * BLK:(pi + 1) * BLK],
                            in_=dsb)

            # ---- pipelined emission ----
            # all ACT stats (tiny Ln/Exp) precede every gram exp stream;
            # each ntp's PE/DVE work hides under the previous gram's exps
            stats("xwA")
            ntp("xwA")
            stats("xwB")
            ntp("xwB")
            stats("ywA")
            stats("ywB")
            stats("yf")
            gram("xA", "xwA", "xwA", cA, 4, d_colsyms[0:1, :], BLK)
            ntp("ywA")
            gram("xB", "xwB", "xwB", cB, 4, d_colsyms[1:2, :], BLK)
            ntp("ywB")
            gram("yA", "ywA", "ywA", cA, 4, d_colsyms[2:3, :], BLK)
            ntp("yf", 0, (n_total // P) // 2)
            gram("yB", "ywB", "ywB", cB, 4, d_colsyms[3:4, :], BLK)
            ntp("yf", (n_total // P) // 2)
            diag_block()
            gram("01", "xwA", "yf", c01, 8, d_colsums01, 0)

            nc.sync.dma_start(out=d_rowsums, in_=rows_sb)

    nc.finalize()
    return nc


_NC_CACHE = {}


def _get_nc(n_total, n_cores):
    key = (n_total, n_cores)
    if key not in _NC_CACHE:
        _NC_CACHE[key] = build(n_total, n_cores)
    return _NC_CACHE[key]


def _window(z, b, nblocks, n_total):
    idx = (np.arange(nblocks * BLK) + b * BLK) % n_total
    return np.ascontiguousarray(z[idx])


def _run(img, mol, trace=False, n_cores=N_CORES):
    img = np.ascontiguousarray(np.asarray(img, dtype=np.float32))
    mol = np.ascontiguousarray(np.asarray(mol, dtype=np.float32))
    n_total = img.shape[0]
    P = 128
    nb, wA, wB, cA, cB, c01, segs, rcols = _layout(n_total)
    nc = _get_nc(n_total, n_cores)

    in_maps = []
    for r in range(n_cores):
        bA, bB = r, nb - 1 - r
        in_maps.append({
            "xwA": _window(img, bA, nb // 2 + 1, n_total),
            "xwB": _window(img, bB, nb // 2, n_total),
            "ywA": _window(mol, bA, nb // 2 + 1, n_total),
            "ywB": _window(mol, bB, nb // 2, n_total),
            "yf": mol,
        })
    res = run_bass_kernel_spmd(nc, in_maps, list(range(n_cores)), trace=trace)
    return _combine(res, n_total, n_cores), res


def _combine(res, n_total, n_cores):
    P = 128
    nb, wA, wB, cA, cB, c01, segs, rcols = _layout(n_total)
    rowsum = np.zeros((3, n_total))
    colsum = np.zeros((3, n_total))
    diags = np.zeros((3, n_total))
    matmap = {"xA": 0, "xB": 0, "yA": 2, "yB": 2}
    for r in range(n_cores):
        bA, bB = r, nb - 1 - r
        out = res.results[r]
        rw = out["rowsums"].astype(np.float64)
        # symmetric-gram row sums
        for seg, borig, ncs_chunks in (("xA", bA, cA), ("xB", bB, cB),
                                       ("yA", bA, cA), ("yB", bB, cB)):
            base, ncs, mc = segs[seg]
            mat = matmap[seg]
            for m in range(mc):
                rows = slice(borig * BLK + m * P, borig * BLK + (m + 1) * P)
                rowsum[mat, rows] += rw[:, base + m * ncs:
                                        base + (m + 1) * ncs].sum(axis=1)
        # sim01 row sums: m<4 -> block bA, m>=4 -> block bB
        base, ncs, mc = segs["01"]
        for m in range(mc):
            borig = bA if m < 4 else bB
            mm = m % 4
            rows = slice(borig * BLK + mm * P, borig * BLK + (mm + 1) * P)
            rowsum[1, rows] += rw[:, base + m * ncs:
                                  base + (m + 1) * ncs].sum(axis=1)
        # symmetric-gram column sums (window-relative -> original cols)
        csym = out["colsyms"].astype(np.float64)
        for row_i, (borig, w) in enumerate(((bA, wA), (bB, wB),
                                            (bA, wA), (bB, wB))):
            mat = 0 if row_i < 2 else 2
            width = w - BLK
            j = np.arange(width)
            orig = ((borig + 1 + j // BLK) % nb) * BLK + j % BLK
            np.add.at(colsum[mat], orig, csym[row_i, :width])
        colsum[1] += out["colsums01"].astype(np.float64)[0]
        # diags: first 512 -> block bA rows, next 512 -> block bB rows
        dg = out["diags"].astype(np.float64)
        for pi, borig in enumerate((bA, bB)):
            rows = slice(borig * BLK, (borig + 1) * BLK)
            diags[:, rows] = dg[:, pi * BLK:(pi + 1) * BLK]

    ed = np.exp(INV_T * diags)
    t00 = rowsum[0] + colsum[0] - ed[0]
    t01r = rowsum[1] - ed[1]
    t01c = colsum[1] - ed[1]
    t11 = rowsum[2] + colsum[2] - ed[2]
    loss = (-INV_T * diags[1]).mean() + 0.5 * (
        np.log(t00) + np.log(t01r) + np.log(t01c) + np.log(t11)).mean()
    return np.array(loss, dtype=np.float32)


def kernel(img_rep, mol_rep):
    loss, _ = _run(img_rep, mol_rep)
    return loss

